# revision 14
# baseline (speedup 1.0000x reference)
"""Depthwise causal Conv1d (K=16) for x:(4, 2048, 8192) f32 on 8 TRN2 NeuronCores.

Strategy (tensor-parallel over channels, no cross-core communication):
  - Each core owns 256 channels (2048 / 8) for all 4 batches.
  - The time axis is cut into overlapping 127-sample windows with stride 112
    (15-sample causal halo), placed on SBUF partitions 0..126 and
    time-REVERSED within each window.  The depthwise conv of one channel is
    then a single banded-Toeplitz matmul on the TensorEngine:
        psum[m, (b,j)] = sum_p A[p, m] * X[p, (b,j)]
        A[p, m]   = w[126 - p - m]              for 111 <= p + m <= 126
        X[p, (b,j)] = x[b, c, 112*j + 111 - p]  (zero for t < 0)
        psum[m, (b,j)] = y[b, c, 112*j + m]
    Only NJ=73 full windows run on device (outputs [0, 8176)); the last 16
    samples per channel and the (identically zero) bias are done on the
    host, so no padded junk window is shipped.
  - All DRAM traffic is bf16 (inputs rounded on host, output upcast on
    host): the problem is HBM-bandwidth bound (16 DMA engines x 25.6 GB/s
    per core), and bf16 halves the bytes while staying ~0.4% rel err
    (budget is 2e-2).  Matmul runs in bf16, PSUM accumulates in f32.
  - DMA queues are dedicated: GpSimd = all loads (pure prefetch), Sync =
    all stores (compute-dependent), so prefetch never queues behind a
    store.  Loads/stores are split per half-chunk for finer pipelining,
    and the last chunks are tapered (16x15 + 8 + 8) to shorten the drain.
  - Epilogue: PSUM(f32) -> SBUF(bf16) converting copies, two channels per
    instruction, alternating Vector / Scalar engines.

The host does the sharding + window-layout transposes with numpy; the device
kernel sees only dense p-major arrays.
"""

import os
import sys

import numpy as np
from numpy.lib.stride_tricks import sliding_window_view

if "/opt/trn_rl_repo" not in sys.path:
    sys.path.insert(0, "/opt/trn_rl_repo")

import ml_dtypes

import concourse.bacc as bacc
import concourse.mybir as mybir
import concourse.tile as tile
from concourse.bass_utils import run_bass_kernel_spmd

F32 = mybir.dt.float32
BF16 = mybir.dt.bfloat16
NP_BF16 = np.dtype(ml_dtypes.bfloat16)
ACT_COPY = mybir.ActivationFunctionType.Copy

N_CORES = 8
B = 4             # batch
DIM = 2048        # channels
T = 8192          # time
K = 16            # conv taps
C = DIM // N_CORES    # channels per core = 256
PIN = 128         # matmul contraction rows (127-sample window + zero row)
PO = 112          # outputs per window (= 127 - 15), multiple of 16
NJ = 73           # full windows per (batch, channel); tail done on host
TD = NJ * PO          # device-computed samples per channel = 8176
XC = B * NJ           # x / out cols per channel = 292
PSB = 512             # psum bank stride (f32 elems); channel pair at 0 / PSB
CHUNKS = [16] * 15 + [8, 8]   # channels per device chunk (tapered tail)

_compiled_nc = None


def _build_kernel():
    nc = bacc.Bacc(None)

    xin = nc.declare_dram_parameter("xin", [PIN, C, XC], BF16, isOutput=False)
    a_in = nc.declare_dram_parameter("a_in", [PIN, C, PO], BF16, isOutput=False)
    yout = nc.declare_dram_parameter("yout", [PO, C, XC], BF16, isOutput=True)

    with tile.TileContext(nc) as tc:
        with (
            tc.tile_pool(name="xpool", bufs=7) as xpool,
            tc.tile_pool(name="apool", bufs=7) as apool,
            tc.tile_pool(name="opool", bufs=6) as opool,
            tc.tile_pool(name="psum", bufs=4, space="PSUM") as pspool,
        ):
            c0 = 0
            for ch in CHUNKS:
                x_t = xpool.tile([PIN, ch * XC], BF16)
                a_t = apool.tile([PIN, ch * PO], BF16)
                o_t = opool.tile([PO, ch * XC], BF16)
                n = ch * XC
                hf = ch // 2          # channels per half
                nh = hf * XC          # cols per half

                # loads (GpSimd queue)
                nc.gpsimd.dma_start(
                    out=x_t[:].rearrange("p (c j) -> p c j", c=ch),
                    in_=xin[:, c0 : c0 + ch, :],
                )
                nc.scalar.dma_start(
                    out=a_t[:].rearrange("p (c m) -> p c m", c=ch),
                    in_=a_in[:, c0 : c0 + ch, :],
                )

                for g in range(ch // 2):
                    ps = pspool.tile([PO, 2 * PSB], F32)
                    for h in range(2):
                        i = 2 * g + h
                        nc.tensor.matmul(
                            ps[:, h * PSB : h * PSB + XC],
                            a_t[:, i * PO : (i + 1) * PO],
                            x_t[:, i * XC : (i + 1) * XC],
                            start=True,
                            stop=True,
                        )
                    # converting psum(f32) -> sbuf(bf16) copy, 2 ch per inst
                    src = ps[:].rearrange("p (g q) -> p g q", g=2)[:, :, 0:XC]
                    dst = o_t[:, 2 * g * XC : (2 * g + 2) * XC].rearrange(
                        "p (g q) -> p g q", g=2
                    )
                    if g % 2 == 0:
                        nc.vector.tensor_copy(dst, src)
                    else:
                        nc.scalar.activation(dst, src, ACT_COPY)

                    # store (Sync queue) as soon as each half's copies land
                    if 2 * (g + 1) == hf:
                        nc.sync.dma_start(
                            out=yout[:, c0 : c0 + hf, :],
                            in_=o_t[:, 0:nh].rearrange("p (c j) -> p c j", c=hf),
                        )
                    elif 2 * (g + 1) == ch:
                        nc.sync.dma_start(
                            out=yout[:, c0 + hf : c0 + ch, :],
                            in_=o_t[:, nh:n].rearrange("p (c j) -> p c j", c=hf),
                        )
                c0 += ch

    nc.compile()
    return nc


def _get_nc():
    global _compiled_nc
    if _compiled_nc is None:
        _compiled_nc = _build_kernel()
    return _compiled_nc


def _prep_core(x, weight, bias, core):
    """Build the per-core input map (numpy only)."""
    cs = slice(core * C, (core + 1) * C)
    xs = x[:, cs, :]                       # [B, C, T]
    w = weight[cs, 0, :]                   # [C, K]

    # X[p, c, (b, j)] = x[b, c, 112*j + 111 - p] for p in [0, 127); row 127 = 0
    # xpad = [15 zeros] ++ x[:TD]; window j = xpad[112j : 112j + 127]
    PW = PIN - 1
    xpad = np.zeros((B, C, PO * (NJ - 1) + PW), dtype=np.float32)
    xpad[:, :, K - 1 :] = xs[:, :, :TD]
    sw = sliding_window_view(xpad, PW, axis=2)[:, :, ::PO, :]    # [B,C,NJ,127]
    xin = np.zeros((PIN, C, B, NJ), dtype=np.float32)
    xin[0:PW] = sw[:, :, :, ::-1].transpose(3, 1, 0, 2)
    xin = np.ascontiguousarray(xin).reshape(PIN, C, XC).astype(NP_BF16)

    # A[p, m] = w[126 - p - m] for 111 <= p + m <= 126, p in [0, 127); row 127 = 0
    idx = np.arange(PW)[:, None] + np.arange(PO)[None, :]    # p + m
    amask = (idx >= 111) & (idx <= 126)
    aidx = np.clip(126 - idx, 0, K - 1)
    a_mat = np.where(amask[None], w[:, aidx], 0.0)           # [C, 127, PO]
    a_in = np.zeros((PIN, C, PO), dtype=np.float32)
    a_in[0:PW] = a_mat.transpose(1, 0, 2)
    a_in = np.ascontiguousarray(a_in).astype(NP_BF16)

    return {"xin": xin, "a_in": a_in}


def run(x, weight, bias, trace=False):
    nc = _get_nc()
    x = np.asarray(x, dtype=np.float32)
    weight = np.asarray(weight, dtype=np.float32)
    bias = np.asarray(bias, dtype=np.float32)
    in_maps = [_prep_core(x, weight, bias, core) for core in range(N_CORES)]
    res = run_bass_kernel_spmd(nc, in_maps, list(range(N_CORES)), trace=trace)

    y = np.empty((B, DIM, T), dtype=np.float32)
    for core in range(N_CORES):
        yp = res.results[core]["yout"].astype(np.float32)        # [PO, C, B*NJ]
        yc = yp.reshape(PO, C, B, NJ).transpose(2, 1, 3, 0)      # [B, C, j, m]
        y[:, core * C : (core + 1) * C, :TD] = yc.reshape(B, C, TD)

    # tail outputs [TD, T) in f32 on host: y[t] = sum_k w[k] x[t - 15 + k]
    wt = weight[:, 0, :]                                         # [DIM, K]
    xt = sliding_window_view(x[:, :, TD - K + 1 :], K, axis=2)   # [B,DIM,16,K]
    y[:, :, TD:] = np.einsum("bcmk,ck->bcm", xt, wt, optimize=True)

    y += bias[None, :, None]
    return y, res


def kernel(x, weight, bias):
    y, _ = run(
        np.asarray(x, dtype=np.float32),
        np.asarray(weight, dtype=np.float32),
        np.asarray(bias, dtype=np.float32),
    )
    return y


# revision 15
# speedup vs baseline: 1.0436x; 1.0436x over previous
"""Depthwise causal Conv1d (K=16) for x:(4, 2048, 8192) f32 on 8 TRN2 NeuronCores.

Strategy (tensor-parallel over channels, no cross-core communication):
  - Each core owns 256 channels (2048 / 8) for all 4 batches.
  - The time axis is cut into overlapping 127-sample windows with stride 112
    (15-sample causal halo), placed on SBUF partitions 0..126 and
    time-REVERSED within each window.  The depthwise conv of one channel is
    then a single banded-Toeplitz matmul on the TensorEngine:
        psum[m, (b,j)] = sum_p A[p, m] * X[p, (b,j)]
        A[p, m]   = w[126 - p - m]              for 111 <= p + m <= 126
        X[p, (b,j)] = x[b, c, 112*j + 111 - p]  (zero for t < 0)
        psum[m, (b,j)] = y[b, c, 112*j + m]
    Only NJ=73 full windows run on device (outputs [0, 8176)); the last 16
    samples per channel and the (identically zero) bias are done on the
    host, so no padded junk window is shipped.
  - All DRAM traffic is bf16 (inputs rounded on host, output upcast on
    host): the problem is HBM-bandwidth bound (16 DMA engines x 25.6 GB/s
    per core), and bf16 halves the bytes while staying ~0.4% rel err
    (budget is 2e-2).  Matmul runs in bf16, PSUM accumulates in f32.
  - DMA queues are dedicated: GpSimd = all loads (pure prefetch), Sync =
    all stores (compute-dependent), so prefetch never queues behind a
    store.  Loads/stores are split per half-chunk for finer pipelining,
    and the last chunks are tapered (16x15 + 8 + 8) to shorten the drain.
  - Epilogue: PSUM(f32) -> SBUF(bf16) converting copies, two channels per
    instruction, alternating Vector / Scalar engines.

The host does the sharding + window-layout transposes with numpy; the device
kernel sees only dense p-major arrays.
"""

import os
import sys

import numpy as np
from numpy.lib.stride_tricks import sliding_window_view

if "/opt/trn_rl_repo" not in sys.path:
    sys.path.insert(0, "/opt/trn_rl_repo")

import ml_dtypes

import concourse.bacc as bacc
import concourse.mybir as mybir
import concourse.tile as tile
from concourse.bass_utils import run_bass_kernel_spmd

F32 = mybir.dt.float32
BF16 = mybir.dt.bfloat16
NP_BF16 = np.dtype(ml_dtypes.bfloat16)
ACT_COPY = mybir.ActivationFunctionType.Copy

N_CORES = 8
B = 4             # batch
DIM = 2048        # channels
T = 8192          # time
K = 16            # conv taps
C = DIM // N_CORES    # channels per core = 256
PIN = 128         # matmul contraction rows (127-sample window + zero row)
PO = 112          # outputs per window (= 127 - 15), multiple of 16
NJ = 73           # full windows per (batch, channel); tail done on host
TD = NJ * PO          # device-computed samples per channel = 8176
XC = B * NJ           # x / out cols per channel = 292
PSB = 512             # psum bank stride (f32 elems); channel pair at 0 / PSB
CHUNKS = [16] * 15 + [8, 8]   # channels per device chunk (tapered tail)

_compiled_nc = None


def _build_kernel():
    nc = bacc.Bacc(None)

    xin = nc.declare_dram_parameter("xin", [PIN, C, XC], BF16, isOutput=False)
    a_in = nc.declare_dram_parameter("a_in", [PIN, C, PO], BF16, isOutput=False)
    yout = nc.declare_dram_parameter("yout", [PO, C, XC], BF16, isOutput=True)

    with tile.TileContext(nc) as tc:
        with (
            tc.tile_pool(name="xpool", bufs=7) as xpool,
            tc.tile_pool(name="apool", bufs=7) as apool,
            tc.tile_pool(name="opool", bufs=6) as opool,
            tc.tile_pool(name="psum", bufs=4, space="PSUM") as pspool,
        ):
            c0 = 0
            for ch in CHUNKS:
                x_t = xpool.tile([PIN, ch * XC], BF16)
                a_t = apool.tile([PIN, ch * PO], BF16)
                o_t = opool.tile([PO, ch * XC], BF16)
                n = ch * XC
                hf = ch // 2          # channels per half
                nh = hf * XC          # cols per half

                # loads (GpSimd queue)
                nc.gpsimd.dma_start(
                    out=x_t[:].rearrange("p (c j) -> p c j", c=ch),
                    in_=xin[:, c0 : c0 + ch, :],
                )
                nc.gpsimd.dma_start(
                    out=a_t[:].rearrange("p (c m) -> p c m", c=ch),
                    in_=a_in[:, c0 : c0 + ch, :],
                )

                for g in range(ch // 2):
                    ps = pspool.tile([PO, 2 * PSB], F32)
                    for h in range(2):
                        i = 2 * g + h
                        nc.tensor.matmul(
                            ps[:, h * PSB : h * PSB + XC],
                            a_t[:, i * PO : (i + 1) * PO],
                            x_t[:, i * XC : (i + 1) * XC],
                            start=True,
                            stop=True,
                        )
                    # converting psum(f32) -> sbuf(bf16) copy, 2 ch per inst
                    src = ps[:].rearrange("p (g q) -> p g q", g=2)[:, :, 0:XC]
                    dst = o_t[:, 2 * g * XC : (2 * g + 2) * XC].rearrange(
                        "p (g q) -> p g q", g=2
                    )
                    if g % 2 == 0:
                        nc.vector.tensor_copy(dst, src)
                    else:
                        nc.scalar.activation(dst, src, ACT_COPY)

                    # store (Sync queue) as soon as each half's copies land
                    if 2 * (g + 1) == hf:
                        nc.sync.dma_start(
                            out=yout[:, c0 : c0 + hf, :],
                            in_=o_t[:, 0:nh].rearrange("p (c j) -> p c j", c=hf),
                        )
                    elif 2 * (g + 1) == ch:
                        nc.sync.dma_start(
                            out=yout[:, c0 + hf : c0 + ch, :],
                            in_=o_t[:, nh:n].rearrange("p (c j) -> p c j", c=hf),
                        )
                c0 += ch

    nc.compile()
    return nc


def _get_nc():
    global _compiled_nc
    if _compiled_nc is None:
        _compiled_nc = _build_kernel()
    return _compiled_nc


def _prep_core(x, weight, bias, core):
    """Build the per-core input map (numpy only)."""
    cs = slice(core * C, (core + 1) * C)
    xs = x[:, cs, :]                       # [B, C, T]
    w = weight[cs, 0, :]                   # [C, K]

    # X[p, c, (b, j)] = x[b, c, 112*j + 111 - p] for p in [0, 127); row 127 = 0
    # xpad = [15 zeros] ++ x[:TD]; window j = xpad[112j : 112j + 127]
    PW = PIN - 1
    xpad = np.zeros((B, C, PO * (NJ - 1) + PW), dtype=np.float32)
    xpad[:, :, K - 1 :] = xs[:, :, :TD]
    sw = sliding_window_view(xpad, PW, axis=2)[:, :, ::PO, :]    # [B,C,NJ,127]
    xin = np.zeros((PIN, C, B, NJ), dtype=np.float32)
    xin[0:PW] = sw[:, :, :, ::-1].transpose(3, 1, 0, 2)
    xin = np.ascontiguousarray(xin).reshape(PIN, C, XC).astype(NP_BF16)

    # A[p, m] = w[126 - p - m] for 111 <= p + m <= 126, p in [0, 127); row 127 = 0
    idx = np.arange(PW)[:, None] + np.arange(PO)[None, :]    # p + m
    amask = (idx >= 111) & (idx <= 126)
    aidx = np.clip(126 - idx, 0, K - 1)
    a_mat = np.where(amask[None], w[:, aidx], 0.0)           # [C, 127, PO]
    a_in = np.zeros((PIN, C, PO), dtype=np.float32)
    a_in[0:PW] = a_mat.transpose(1, 0, 2)
    a_in = np.ascontiguousarray(a_in).astype(NP_BF16)

    return {"xin": xin, "a_in": a_in}


def run(x, weight, bias, trace=False):
    nc = _get_nc()
    x = np.asarray(x, dtype=np.float32)
    weight = np.asarray(weight, dtype=np.float32)
    bias = np.asarray(bias, dtype=np.float32)
    in_maps = [_prep_core(x, weight, bias, core) for core in range(N_CORES)]
    res = run_bass_kernel_spmd(nc, in_maps, list(range(N_CORES)), trace=trace)

    y = np.empty((B, DIM, T), dtype=np.float32)
    for core in range(N_CORES):
        yp = res.results[core]["yout"].astype(np.float32)        # [PO, C, B*NJ]
        yc = yp.reshape(PO, C, B, NJ).transpose(2, 1, 3, 0)      # [B, C, j, m]
        y[:, core * C : (core + 1) * C, :TD] = yc.reshape(B, C, TD)

    # tail outputs [TD, T) in f32 on host: y[t] = sum_k w[k] x[t - 15 + k]
    wt = weight[:, 0, :]                                         # [DIM, K]
    xt = sliding_window_view(x[:, :, TD - K + 1 :], K, axis=2)   # [B,DIM,16,K]
    y[:, :, TD:] = np.einsum("bcmk,ck->bcm", xt, wt, optimize=True)

    y += bias[None, :, None]
    return y, res


def kernel(x, weight, bias):
    y, _ = run(
        np.asarray(x, dtype=np.float32),
        np.asarray(weight, dtype=np.float32),
        np.asarray(bias, dtype=np.float32),
    )
    return y
